# revision 32
# baseline (speedup 1.0000x reference)
"""Trainium2 Bass kernel for NemotronFlash Mamba decoder layer (v3).

Sharding: 8 cores = 2 batches x 4 sequence shards of 512 tokens.
All compute is shard-local except the SSD inter-chunk state, which is
exchanged via one AllGather of (L_k, D_k) within each 4-core batch group.

Design notes:
- in-projection emits xBC/dt tiles first, z tiles last; the depthwise conv
  runs on the tensor engine (diagonal stationaries) and overlaps it.
- SSD Y is computed feature-major (Y^T): per head pair one PSUM tile
  accumulates Y_diag + Y_off (odd head written at partition offset 64);
  the D*x skip folds into the PSUM drain and the gated-norm multiply and
  square-reduce fold into the same chunk-major loop. No y transposes.
- decay matrices (mt) and Ct are produced chunk-by-chunk in small rolling
  buffers inside the Y loop; the scheduler hoists them into the
  collective-wait window.
- SBUF is one workspace pool with tag-chained slot reuse (LIFO-safe).
"""
import sys
import numpy as np

sys.path.insert(0, "/opt/trn_rl_repo")

from contextlib import ExitStack  # noqa: E402
import ml_dtypes  # noqa: E402
import concourse.bass as bass  # noqa: E402
import concourse.mybir as mybir  # noqa: E402
import concourse.tile as tile  # noqa: E402
from concourse import bacc  # noqa: E402
from concourse.bass_utils import run_bass_kernel_spmd  # noqa: E402

F32 = mybir.dt.float32
BF16 = mybir.dt.bfloat16
AF = mybir.ActivationFunctionType
OP = mybir.AluOpType

H = 1024
E = 2048
NH = 32
P = 64
NST = 128          # d_state
KC = 4             # d_conv
Q = 128            # chunk len
FF = 4096
CONV = E + 2 * NST          # 2304
D_IN = 2 * E + 2 * NST + NH  # 4384
EPS = 1e-6
NEPS = 1e-5
LSEQ = 512         # tokens per shard
NCHUNK = LSEQ // Q  # 4
NROW = 5           # 5 row tiles of 128 = 640 padded rows
HALO = 3
NCORES = 8

NZT = E // Q       # 16 z tiles
NXT = CONV // Q    # 18 xBC tiles
NMT = 35           # in-proj M tiles (16 z + 18 xBC + 1 dt)
NKH = H // Q       # 8 k tiles over H
NKE = E // Q       # 16 k tiles over E
NFT = FF // Q      # 32 FF tiles


def row_bcast(ap_row, parts=128):
    """AP broadcasting a [1, n] row across `parts` partitions (step-0)."""
    return bass.AP(tensor=ap_row.tensor, offset=ap_row.offset,
                   ap=[[0, parts]] + [list(x) for x in ap_row.ap[1:]])


def colbc(src_ap, n, rep):
    # [128, n, rep] broadcast of per-head columns along a new inner axis
    return bass.AP(tensor=src_ap.tensor, offset=src_ap.offset,
                   ap=[list(src_ap.ap[0])] + [[1, n], [0, rep]])


def rowbc(src_ap, rep, n):
    # [128, rep, n] broadcast of a [128, n] tile along the middle axis
    return bass.AP(tensor=src_ap.tensor, offset=src_ap.offset,
                   ap=[list(src_ap.ap[0])] + [[0, rep], [1, n]])


def build_program(dvals):
    nc = bacc.Bacc("TRN2", target_bir_lowering=False, debug=False,
                   num_devices=NCORES)

    hs_in = nc.dram_tensor("hs", [NROW * 128, H], F32, kind="ExternalInput")
    wiT = nc.dram_tensor("wiT", [9 * 128, NKH * 512], BF16,
                         kind="ExternalInput")
    woT = nc.dram_tensor("woT", [E, H], BF16, kind="ExternalInput")
    wgT = nc.dram_tensor("wgT", [NFT * 128, NKH * 128], BF16,
                         kind="ExternalInput")
    wuT = nc.dram_tensor("wuT", [NFT * 128, NKH * 128], BF16,
                         kind="ExternalInput")
    wdT = nc.dram_tensor("wdT", [FF, H], BF16, kind="ExternalInput")
    wconvd = nc.dram_tensor("wconvd", [128, NXT * KC * 128], BF16,
                            kind="ExternalInput")
    bconv = nc.dram_tensor("bconv", [128, NXT], F32, kind="ExternalInput")
    avec = nc.dram_tensor("avec", [NH, 1], F32, kind="ExternalInput")
    dtb = nc.dram_tensor("dtb", [NH, 1], F32, kind="ExternalInput")
    mask8 = nc.dram_tensor("mask8", [128, 8], F32, kind="ExternalInput")
    negmask = nc.dram_tensor("negmask", [128, 128], F32, kind="ExternalInput")
    idf32 = nc.dram_tensor("idf32", [128, 128], F32, kind="ExternalInput")
    dcol_in = nc.dram_tensor("dcol", [128, NZT], F32, kind="ExternalInput")
    out_d = nc.dram_tensor("out", [LSEQ, H], F32, kind="ExternalOutput")

    with tile.TileContext(nc) as tc, ExitStack() as stack:
        consts = stack.enter_context(tc.tile_pool(name="consts", bufs=1))
        bconv_sb = consts.tile([128, NXT], F32)
        nc.sync.dma_start(out=bconv_sb[:], in_=bconv[:])
        avec_sb = consts.tile([NH, 1], F32)
        nc.sync.dma_start(out=avec_sb[:], in_=avec[:])
        dtb_sb = consts.tile([NH, 1], F32)
        nc.sync.dma_start(out=dtb_sb[:], in_=dtb[:])
        mask_sb = consts.tile([128, 8], F32)
        nc.sync.dma_start(out=mask_sb[:], in_=mask8[:])
        nm_sb = consts.tile([128, 128], F32)
        nc.sync.dma_start(out=nm_sb[:], in_=negmask[:])
        id_sb = consts.tile([128, 128], F32)
        nc.sync.dma_start(out=id_sb[:], in_=idf32[:])
        dcol_sb = consts.tile([128, NZT], F32)
        nc.sync.dma_start(out=dcol_sb[:], in_=dcol_in[:])
        ones_bf = consts.tile([128, 1], BF16)
        nc.vector.memset(ones_bf[:], 1.0)
        zero32 = consts.tile([NH, Q], F32)
        nc.vector.memset(zero32[:], 0.0)
        epsc = consts.tile([128, 1], F32)
        nc.vector.memset(epsc[:], EPS)
        nepsc = consts.tile([128, 1], F32)
        nc.vector.memset(nepsc[:], NEPS)

        ccdram = stack.enter_context(
            tc.tile_pool(name="ccdram", bufs=1, space="DRAM"))
        cc_in = ccdram.tile([128, E + 1], BF16)
        cc_out = ccdram.tile([4, 128, E + 1], BF16)
        acsR_d = ccdram.tile([NCHUNK * NH, Q], F32)
        mt_d = ccdram.tile([128, NCHUNK * NH * Q], BF16)
        ct_d = ccdram.tile([128, NCHUNK * NH * Q], BF16)
        acst_d = ccdram.tile([1, NCHUNK * 2 * NH], F32)
        drow_d = ccdram.tile([1, 4 * NH], F32)
        rs_d = ccdram.tile([1, LSEQ], BF16)

        # Workspace pool; tag-chained slot reuse (bufs=1 per tag):
        #   szs: szT(16K)
        #   xcs: xc(16K) -> wo_h1(16K)
        #   hts: hT(10K) -> gt(16K)
        #   xbs: xbc(18.5K) -> S_sb(16K) -> h2nT(8K)
        #   csy: cstates(16K) -> gu(32K)
        #   xth: x_tm(16K)+B_tm via cf -> h2(16K f32)
        #   xdw: xdt_all(16K) -> wo_h0(16K)
        ws = stack.enter_context(tc.tile_pool(name="ws", bufs=1))
        szT = ws.tile([128, NZT, LSEQ], BF16, tag="szs")
        xc = ws.tile([128, NZT, LSEQ], BF16, tag="xcs")
        hT = ws.tile([128, NKH, NROW * 128], BF16, tag="hts")

        es_cf = ExitStack()                        # C .. end of Y loop
        pCF = es_cf.enter_context(tc.tile_pool(name="pCF", bufs=1))
        dtacsT = pCF.tile([128, NCHUNK, 2 * NH], F32)
        alast = pCF.tile([128, NCHUNK, NH], F32)
        wdtb = pCF.tile([128, NCHUNK, NH], BF16)
        dtb16 = pCF.tile([128, NCHUNK, NH], BF16)
        dcstb = pCF.tile([128, NCHUNK, NH], BF16)
        dt_sb = pCF.tile([NH, LSEQ], F32)
        acs = pCF.tile([NH, LSEQ], F32)
        dtraw = pCF.tile([NH, LSEQ], F32)
        xcbc = pCF.tile([128, 2, LSEQ], BF16)
        G_sb = pCF.tile([128, NCHUNK, Q], BF16)
        B_tm = pCF.tile([128, NCHUNK, NST], BF16)

        # ---------------- Phase A: rmsnorm1 + h^T ----------------
        # batched per ACT function to avoid table reloads
        with tc.tile_pool(name="pA", bufs=1) as pA, \
             tc.tile_pool(name="stat", bufs=1) as stat:
            hsts, ssums, rss = [], [], []
            for r in range(NROW):
                hst = pA.tile([128, H], F32, tag=f"hst{r}", name=f"hst{r}")
                nc.sync.dma_start(out=hst[:], in_=hs_in[r * 128:(r + 1) * 128, :])
                hsts.append(hst)
            for r in range(NROW):
                sq = pA.tile([128, H], F32, tag="sq", bufs=2)
                ssum = stat.tile([128, 1], F32, tag=f"ssum{r}",
                                 name=f"ssum{r}")
                nc.scalar.activation(out=sq[:], in_=hsts[r][:],
                                     func=AF.Square, accum_out=ssum[:])
                ssums.append(ssum)
            for r in range(NROW):
                rs = stat.tile([128, 1], F32, tag=f"rs{r}", name=f"rs{r}")
                nc.scalar.activation(out=rs[:], in_=ssums[r][:],
                                     func=AF.Ln, scale=1.0 / H,
                                     bias=epsc[:])
                rss.append(rs)
            for r in range(NROW):
                nc.scalar.activation(out=rss[r][:], in_=rss[r][:],
                                     func=AF.Exp, scale=-0.5)
            for r in range(NROW):
                hbf = pA.tile([128, H], BF16, tag="hbf", bufs=2)
                nc.vector.tensor_scalar_mul(hbf[:], hsts[r][:], rss[r][:])
                eng = nc.sync if r % 2 == 0 else nc.scalar
                eng.dma_start_transpose(
                    hT[:, :, r * 128:(r + 1) * 128], hbf[:])

        cstates = ws.tile([128, NCHUNK, E], BF16, tag="csy")
        x_tm = ws.tile([128, NCHUNK, E], BF16, tag="xth")
        xbc = ws.tile([128, NXT, HALO + LSEQ], BF16, tag="xbs")

        # ---------------- Phase B: in-projection ----------------
        groups = []
        m = 0
        while m < NMT:
            g0 = m
            cols = 0
            while m < NMT and cols + (32 if m == NMT - 1 else 128) <= 512:
                cols += (32 if m == NMT - 1 else 128)
                m += 1
            groups.append((g0, m, cols))

        def do_group(gi, wip, psB, psBh):
            g0, g1, cols = groups[gi]
            wi_g = wip.tile([128, NKH, 512], BF16, tag="wi")
            base = g0 * 128
            nc.sync.dma_start(
                out=wi_g[:],
                in_=wiT[gi * 128:(gi + 1) * 128, :])
            for mm in range(g0, g1):
                mrows = 32 if mm == NMT - 1 else 128
                moff = mm * 128 - base
                ps = psB.tile([128, LSEQ], F32, tag="ps")
                for k in range(NKH):
                    nc.tensor.matmul(
                        ps[:mrows, :],
                        wi_g[:, k, moff:moff + mrows],
                        hT[:, k, HALO:HALO + LSEQ],
                        start=(k == 0), stop=(k == NKH - 1))
                if mm < NZT:
                    nc.vector.tensor_copy(szT[:, mm, :], ps[:])
                elif mm < NZT + NXT:
                    j = mm - NZT
                    nc.vector.tensor_copy(xbc[:, j, HALO:], ps[:])
                    psh = psBh.tile([128, HALO], F32, tag="psh")
                    for k in range(NKH):
                        nc.tensor.matmul(
                            psh[:], wi_g[:, k, moff:moff + 128],
                            hT[:, k, 0:HALO],
                            start=(k == 0), stop=(k == NKH - 1))
                    nc.vector.tensor_copy(xbc[:, j, 0:HALO], psh[:])
                else:
                    nc.vector.tensor_copy(dtraw[:], ps[:NH, :])

        with tc.tile_pool(name="wip", bufs=2) as wip, \
             tc.tile_pool(name="psB", bufs=2, space="PSUM") as psB, \
             tc.tile_pool(name="psBh", bufs=1, space="PSUM") as psBh:
            for gi in (4, 5, 6, 7, 8):
                do_group(gi, wip, psB, psBh)

            # ------------- Phase C: dt path -------------
            with tc.tile_pool(name="pC", bufs=2) as pC, \
                 tc.tile_pool(name="psC", bufs=2, space="PSUM") as psC:
                nc.scalar.activation(out=dtraw[:], in_=dtraw[:],
                                     func=AF.Exp, bias=dtb_sb[:])
                nc.vector.tensor_scalar_add(dtraw[:], dtraw[:], 1.0)
                nc.scalar.activation(out=dt_sb[:], in_=dtraw[:],
                                     func=AF.Ln)
                nc.vector.tensor_scalar_mul(dtraw[:], dt_sb[:],
                                            avec_sb[:])
                for c in range(NCHUNK):
                    nc.vector.tensor_tensor_scan(
                        acs[:, c * Q:(c + 1) * Q],
                        dtraw[:, c * Q:(c + 1) * Q],
                        zero32[:], 0.0, OP.add, OP.add)
                av = acsR_d[:]
                nc.sync.dma_start(
                    out=bass.AP(tensor=av.tensor, offset=av.offset,
                                ap=[[Q, NH], [NH * Q, NCHUNK], [1, Q]]),
                    in_=acs[:].rearrange("h (c q) -> h c q", c=NCHUNK))
                for c in range(NCHUNK):
                    pst = psC.tile([128, NH], F32, tag="pst")
                    nc.tensor.transpose(pst[:],
                                        dt_sb[:, c * Q:(c + 1) * Q],
                                        id_sb[0:NH, 0:NH])
                    nc.scalar.copy(dtacsT[:, c, 0:NH], pst[:])
                    pst2 = psC.tile([128, NH], F32, tag="pst2")
                    nc.tensor.transpose(pst2[:],
                                        acs[:, c * Q:(c + 1) * Q],
                                        id_sb[0:NH, 0:NH])
                    nc.scalar.copy(dtacsT[:, c, NH:2 * NH], pst2[:])
                nc.sync.dma_start(out=acst_d[:],
                                  in_=dtacsT[127:128, :, :])
                at_ = acst_d[:]
                nc.sync.dma_start(
                    out=alast[:],
                    in_=bass.AP(tensor=at_.tensor, offset=at_.offset + NH,
                                ap=[[0, 128], [2 * NH, NCHUNK], [1, NH]]))
                dec0 = pC.tile([128, NCHUNK, NH], F32, tag="dec0")
                nc.vector.scalar_tensor_tensor(
                    out=dec0[:], in0=dtacsT[:, :, NH:2 * NH], scalar=-1.0,
                    in1=alast[:], op0=OP.mult, op1=OP.add)
                decT = pC.tile([128, NCHUNK, NH], F32, tag="decT")
                nc.scalar.activation(out=decT[:], in_=dec0[:], func=AF.Exp)
                nc.vector.tensor_mul(wdtb[:], decT[:], dtacsT[:, :, 0:NH])
                nc.vector.tensor_copy(dtb16[:], dtacsT[:, :, 0:NH])
                nc.scalar.activation(out=dcstb[:], in_=alast[:], func=AF.Exp)

            # ------------- Phase D: conv on tensor engine -------------
            with tc.tile_pool(name="wcd", bufs=3) as wcd, \
                 tc.tile_pool(name="psD", bufs=2, space="PSUM") as psD:
                for j in range(NXT):
                    wc_j = wcd.tile([128, KC, 128], BF16, tag="wc")
                    nc.sync.dma_start(
                        out=wc_j[:],
                        in_=wconvd[:, j * KC * 128:(j + 1) * KC * 128])
                    psc = psD.tile([128, LSEQ], F32, tag="psc")
                    for k in range(KC):
                        nc.tensor.matmul(
                            psc[:], wc_j[:, k, :], xbc[:, j, k:k + LSEQ],
                            start=(k == 0), stop=(k == KC - 1))
                    xdst = (xc[:, j, :] if j < NZT
                            else xcbc[:, j - NZT, :])
                    nc.scalar.activation(out=xdst, in_=psc[:],
                                         func=AF.Silu,
                                         bias=bconv_sb[:, j:j + 1])
                    if j < NZT:
                        eng = nc.sync if j % 2 == 0 else nc.scalar
                        eng.dma_start_transpose(
                            x_tm[:, :, j * 128:(j + 1) * 128],
                            xc[:, j, :])
                    elif j == NZT:
                        nc.sync.dma_start_transpose(
                            B_tm[:], xcbc[:, 0, :])
            # G gram matrices (needs xcbc from conv j=16,17)
            with tc.tile_pool(name="psGm", bufs=2, space="PSUM") as psGm:
                for c in range(NCHUNK):
                    gps = psGm.tile([128, Q], F32, tag="gps")
                    nc.tensor.matmul(gps[:], xcbc[:, 0, c * Q:(c + 1) * Q],
                                     xcbc[:, 1, c * Q:(c + 1) * Q],
                                     start=True, stop=True)
                    nc.vector.tensor_mul(G_sb[:, c, :], gps[:], nm_sb[:])

            # ------------ Phase E: states + collective (before z) --------
            xv = [x_tm[:, c, :].rearrange("p (h q) -> p h q", h=NH)
                  for c in range(NCHUNK)]
            HG = 8
            NG = NH // HG
            es_pe = ExitStack()
            psE = es_pe.enter_context(
                tc.tile_pool(name="psE", bufs=2, space="PSUM"))
            pE = es_pe.enter_context(tc.tile_pool(name="pE", bufs=3))
            for g in range(NG):
                for c in range(NCHUNK):
                    xdd = pE.tile([128, HG, P], BF16, tag="xdd")
                    nc.vector.tensor_mul(
                        xdd[:],
                        x_tm[:, c, g * 512:(g + 1) * 512].rearrange(
                            "p (h q) -> p h q", h=HG),
                        colbc(wdtb[:, c, g * HG:(g + 1) * HG], HG, P))
                    ps_st = psE.tile([128, 512], F32, tag="ps_st")
                    nc.tensor.matmul(
                        ps_st[:], B_tm[:, c, :], xdd[:],
                        start=True, stop=True)
                    nc.vector.tensor_copy(
                        cstates[:, c, g * 512:(g + 1) * 512], ps_st[:])
            # L combine from zero init (bf16, in-place accumulator)
            Lacc = pE.tile([128, E], BF16, tag="lacc", bufs=1)
            nc.vector.tensor_copy(Lacc[:], cstates[:, 0, :])
            for c in range(1, NCHUNK):
                nc.vector.tensor_mul(
                    Lacc[:].rearrange("p (h q) -> p h q", h=NH),
                    Lacc[:].rearrange("p (h q) -> p h q", h=NH),
                    colbc(dcstb[:, c, :], NH, P))
                nc.vector.tensor_add(
                    Lacc[:].rearrange("p (h q) -> p h q", h=NH),
                    Lacc[:].rearrange("p (h q) -> p h q", h=NH),
                    cstates[:, c, :].rearrange("p (h q) -> p h q", h=NH))
            acs4 = acs[:].rearrange("p (c q) -> p c q", c=NCHUNK)[:, :, Q - 1]
            asum = pE.tile([NH, 1], F32, tag="asum")
            nc.vector.tensor_reduce(asum[:], acs4, axis=mybir.AxisListType.X,
                                    op=OP.add)
            dkcol = pE.tile([NH, 1], BF16, tag="dkcol")
            nc.scalar.activation(out=dkcol[:], in_=asum[:], func=AF.Exp)
            nc.gpsimd.dma_start(out=cc_in[:, 0:E], in_=Lacc[:])
            nc.gpsimd.dma_start(out=cc_in[0:NH, E:E + 1], in_=dkcol[:])
            nc.gpsimd.collective_compute(
                "AllGather", OP.bypass,
                replica_groups=[[0, 1, 2, 3], [4, 5, 6, 7]],
                ins=[cc_in.opt()], outs=[cc_out.opt()])
            es_pe.close()

            # z tiles last (needed only at gating)
            for gi in (0, 1, 2, 3):
                do_group(gi, wip, psB, psBh)

        # ---------------- xdt build (pre-collective) ----------------
        xdt_all = ws.tile([128, NCHUNK, E], BF16, tag="xdw")
        for c in range(NCHUNK):
            nc.vector.tensor_mul(
                xdt_all[:, c, :].rearrange("p (h q) -> p h q", h=NH),
                xv[c], colbc(dtb16[:, c, :], NH, P))

        # ------- mt/Ct prep (pre-collective) -> DRAM staging ----------
        HQ = 8
        with tc.tile_pool(name="acsbc", bufs=1) as acsbcp, \
             tc.tile_pool(name="segp", bufs=2) as segp, \
             tc.tile_pool(name="eLp", bufs=2) as eLp, \
             tc.tile_pool(name="mtcp", bufs=2) as mtcp, \
             tc.tile_pool(name="ctcp", bufs=2) as ctcp:
            for c in range(NCHUNK):
                mt_c = mtcp.tile([128, NH, Q], BF16, tag="mtc")
                Ct_c = ctcp.tile([128, NH, Q], BF16, tag="ctc")
                for half in range(2):
                    hb = half * 16
                    ab = acsbcp.tile([128, 16, Q], F32, tag="ab")
                    av2 = acsR_d[:]
                    nc.sync.dma_start(
                        out=ab[:],
                        in_=bass.AP(tensor=av2.tensor,
                                    offset=av2.offset + (c * NH + hb) * Q,
                                    ap=[[0, 128], [1, 16 * Q]]))
                    for hi in range(16 // HQ):
                        h0 = hi * HQ
                        # seg = max(acs_s - acs_q, 0); eL = exp(-seg)
                        seg = segp.tile([128, HQ, Q], F32, tag="seg")
                        nc.vector.scalar_tensor_tensor(
                            out=seg[:], in0=ab[:, h0:h0 + HQ, :],
                            scalar=-1.0,
                            in1=colbc(
                                dtacsT[:, c,
                                       NH + hb + h0:NH + hb + h0 + HQ],
                                HQ, Q),
                            op0=OP.mult, op1=OP.add)
                        nc.vector.tensor_scalar_max(seg[:], seg[:], 0.0)
                        eL = eLp.tile([128, HQ, Q], BF16, tag="eL")
                        nc.scalar.activation(out=eL[:], in_=seg[:],
                                             func=AF.Exp, scale=-1.0)
                        nc.vector.tensor_mul(
                            mt_c[:, hb + h0:hb + h0 + HQ, :], eL[:],
                            rowbc(G_sb[:, c, :], HQ, Q))
                    # Ct half: exp(acs_q) from the same broadcast
                    nc.scalar.activation(out=Ct_c[:, hb:hb + 16, :],
                                         in_=ab[:], func=AF.Exp)
                nc.vector.tensor_mul(
                    Ct_c[:], Ct_c[:],
                    rowbc(xcbc[:, 1, c * Q:(c + 1) * Q], NH, Q))
                nc.scalar.dma_start(
                    out=mt_d[:, c * NH * Q:(c + 1) * NH * Q], in_=mt_c[:])
                nc.sync.dma_start(
                    out=ct_d[:, c * NH * Q:(c + 1) * NH * Q], in_=Ct_c[:])

        # z silu (batched; needed only at gating)
        for mz in range(NZT):
            nc.scalar.activation(out=szT[:, mz, :], in_=szT[:, mz, :],
                                 func=AF.Silu)

        # ---------------- S_init combine + S recurrence ----------------
        S_sb = ws.tile([128, NCHUNK, E], BF16, tag="xbs")
        with tc.tile_pool(name="pS", bufs=1) as pS:
            Lg = pS.tile([128, 4, E], BF16, tag="Lg")
            Dg = pS.tile([NH, 4], BF16, tag="Dg")
            for jj in range(4):
                nc.sync.dma_start(out=Dg[:, jj:jj + 1],
                                  in_=cc_out[jj, 0:NH, E:E + 1])
            deff = pS.tile([NH, 4], F32, tag="deff")
            for jj in range(4):
                nc.vector.scalar_tensor_tensor(
                    out=deff[:, jj:jj + 1], in0=Dg[:, jj:jj + 1],
                    scalar=mask_sb[0:NH, jj:jj + 1],
                    in1=mask_sb[0:NH, 4 + jj:5 + jj],
                    op0=OP.mult, op1=OP.add)
            for jj in range(4):
                nc.sync.dma_start(out=drow_d[0:1, jj * NH:(jj + 1) * NH],
                                  in_=deff[:, jj:jj + 1])
            dbc = pS.tile([128, 4 * NH], F32, tag="dbc")
            nc.sync.dma_start(out=dbc[:], in_=row_bcast(drow_d[0:1, :]))
            for jj in range(4):
                nc.sync.dma_start(out=Lg[:, jj, :], in_=cc_out[jj, :, 0:E])
            Sacc = pS.tile([128, E], BF16, tag="sacc", bufs=1)
            nc.vector.tensor_scalar_mul(Sacc[:], Lg[:, 0, :], mask_sb[:, 0:1])
            for jj in range(1, 4):
                nc.vector.tensor_mul(
                    Sacc[:].rearrange("p (h q) -> p h q", h=NH),
                    Sacc[:].rearrange("p (h q) -> p h q", h=NH),
                    colbc(dbc[:, jj * NH:(jj + 1) * NH], NH, P))
                nc.vector.scalar_tensor_tensor(
                    out=Sacc[:], in0=Lg[:, jj, :],
                    scalar=mask_sb[:, jj:jj + 1], in1=Sacc[:],
                    op0=OP.mult, op1=OP.add)
            nc.vector.tensor_copy(S_sb[:, 0, :], Sacc[:])
            # S recurrence
            for c in range(NCHUNK - 1):
                nc.vector.tensor_mul(
                    S_sb[:, c + 1, :].rearrange("p (h q) -> p h q", h=NH),
                    S_sb[:, c, :].rearrange("p (h q) -> p h q", h=NH),
                    colbc(dcstb[:, c, :], NH, P))
                nc.vector.tensor_add(
                    S_sb[:, c + 1, :], S_sb[:, c + 1, :], cstates[:, c, :])

        # ------- Y: chunk-major mms + drains + gating (feature-major) ----
        gt = ws.tile([128, NKE, LSEQ], BF16, tag="hts")
        with tc.tile_pool(name="mtsp", bufs=2) as mtsp, \
             tc.tile_pool(name="ctsp", bufs=2) as ctsp, \
             tc.tile_pool(name="psY", bufs=6, space="PSUM") as psY, \
             tc.tile_pool(name="psN", bufs=1, space="PSUM") as psN, \
             tc.tile_pool(name="pYd", bufs=3) as pYd:
            sqps = psN.tile([128, LSEQ], F32)
            for c in range(NCHUNK):
                mt_c = mtsp.tile([128, NH, Q], BF16, tag="mts")
                nc.scalar.dma_start(
                    out=mt_c[:], in_=mt_d[:, c * NH * Q:(c + 1) * NH * Q])
                Ct_c = ctsp.tile([128, NH, Q], BF16, tag="cts")
                nc.sync.dma_start(
                    out=Ct_c[:], in_=ct_d[:, c * NH * Q:(c + 1) * NH * Q])
                for j in range(NZT):
                    psy = psY.tile([128, Q], F32, tag="psy")
                    for hh in range(2):
                        h = 2 * j + hh
                        out_ap = psy[hh * 64:(hh + 1) * 64, :]
                        nc.tensor.matmul(
                            out_ap,
                            xdt_all[:, c, h * P:(h + 1) * P],
                            mt_c[:, h, :],
                            start=True, stop=False)
                        nc.tensor.matmul(
                            out_ap,
                            S_sb[:, c, h * P:(h + 1) * P],
                            Ct_c[:, h, :],
                            start=False, stop=True)
                    ydr = pYd.tile([128, Q], BF16, tag="ydr")
                    nc.vector.scalar_tensor_tensor(
                        out=ydr[:], in0=xc[:, j, c * Q:(c + 1) * Q],
                        scalar=dcol_sb[:, j:j + 1], in1=psy[:],
                        op0=OP.mult, op1=OP.add)
                    nc.vector.tensor_mul(gt[:, j, c * Q:(c + 1) * Q], ydr[:],
                                         szT[:, j, c * Q:(c + 1) * Q])
                    g2 = pYd.tile([128, Q], BF16, tag="g2")
                    nc.scalar.activation(out=g2[:],
                                         in_=gt[:, j, c * Q:(c + 1) * Q],
                                         func=AF.Square)
                    nc.tensor.matmul(sqps[0:1, c * Q:(c + 1) * Q],
                                     ones_bf[:], g2[:],
                                     start=(j == 0), stop=(j == NZT - 1))
            rsrow = pYd.tile([1, LSEQ], F32, tag="rsrow")
            nc.scalar.activation(out=rsrow[:], in_=sqps[0:1, :], func=AF.Ln,
                                 scale=1.0 / E, bias=nepsc[0:1, :])
            rsbf = pYd.tile([1, LSEQ], BF16, tag="rsbf")
            nc.scalar.activation(out=rsbf[:], in_=rsrow[:], func=AF.Exp,
                                 scale=-0.5)
            nc.sync.dma_start(out=rs_d[:], in_=rsbf[:])
        es_cf.close()

        # ---------------- Phase G: norm + out-proj + rms2 ----------------
        h2 = ws.tile([128, NCHUNK, H], F32, tag="xth")
        with tc.tile_pool(name="pGa", bufs=2) as pGa, \
             tc.tile_pool(name="psO", bufs=3, space="PSUM") as psO, \
             tc.tile_pool(name="stat2", bufs=4) as stat2:
            rsbc = pGa.tile([128, LSEQ], BF16, tag="rsbc", bufs=1)
            nc.sync.dma_start(out=rsbc[:], in_=row_bcast(rs_d[0:1, :]))
            for mz in range(NKE):
                nc.vector.tensor_mul(gt[:, mz, :], gt[:, mz, :], rsbc[:])
            for half in range(2):
                wo_h = ws.tile([128, NKE, 512], BF16,
                               tag=("xdw" if half == 0 else "xcs"),
                               name=f"wo_h{half}")
                for k in range(NKE):
                    nc.sync.dma_start(
                        out=wo_h[:, k, :],
                        in_=woT[k * 128:(k + 1) * 128,
                                half * 512:(half + 1) * 512])
                for tt in range(NCHUNK):
                    ps = psO.tile([128, 512], F32, tag="po")
                    for k in range(NKE):
                        nc.tensor.matmul(
                            ps[:], gt[:, k, tt * 128:(tt + 1) * 128],
                            wo_h[:, k, :],
                            start=(k == 0), stop=(k == NKE - 1))
                    hsr = pGa.tile([128, 512], F32, tag="hsr")
                    nc.sync.dma_start(
                        out=hsr[:],
                        in_=hs_in[HALO + tt * 128:HALO + (tt + 1) * 128,
                                  half * 512:(half + 1) * 512])
                    nc.vector.tensor_add(
                        h2[:, tt, half * 512:(half + 1) * 512],
                        ps[:], hsr[:])
            # rms2 + transpose (batched per ACT function)
            h2nT = ws.tile([128, NKH, LSEQ], BF16, tag="xbs")
            ss2s, rs2s = [], []
            for tt in range(NCHUNK):
                sq2 = pGa.tile([128, H], F32, tag="sq2")
                ss2 = stat2.tile([128, 1], F32, tag=f"ss2{tt}",
                                 name=f"ss2{tt}")
                nc.scalar.activation(out=sq2[:], in_=h2[:, tt, :],
                                     func=AF.Square, accum_out=ss2[:])
                ss2s.append(ss2)
            for tt in range(NCHUNK):
                rs2 = stat2.tile([128, 1], F32, tag=f"rs2{tt}",
                                 name=f"rs2{tt}")
                nc.scalar.activation(out=rs2[:], in_=ss2s[tt][:],
                                     func=AF.Ln, scale=1.0 / H,
                                     bias=epsc[:])
                rs2s.append(rs2)
            for tt in range(NCHUNK):
                nc.scalar.activation(out=rs2s[tt][:], in_=rs2s[tt][:],
                                     func=AF.Exp, scale=-0.5)
            for tt in range(NCHUNK):
                h2n = pGa.tile([128, H], BF16, tag="h2n")
                nc.vector.tensor_scalar_mul(h2n[:], h2[:, tt, :], rs2s[tt][:])
                eng = nc.sync if tt % 2 == 0 else nc.scalar
                eng.dma_start_transpose(
                    h2nT[:, :, tt * 128:(tt + 1) * 128], h2n[:])

        # ---------------- Phase H: MLP ----------------
        gu = ws.tile([128, NFT, LSEQ], BF16, tag="csy")
        with tc.tile_pool(name="wmP", bufs=3) as wmP, \
             tc.tile_pool(name="psM", bufs=4, space="PSUM") as psM, \
             tc.tile_pool(name="pM", bufs=3) as pM:
            for mf in range(NFT):
                wg_m = wmP.tile([128, NKH, 128], BF16, tag="wg")
                nc.sync.dma_start(out=wg_m[:],
                                  in_=wgT[mf * 128:(mf + 1) * 128, :])
                wu_m = wmP.tile([128, NKH, 128], BF16, tag="wu")
                nc.sync.dma_start(out=wu_m[:],
                                  in_=wuT[mf * 128:(mf + 1) * 128, :])
                gps = psM.tile([128, LSEQ], F32, tag="gps")
                for k in range(NKH):
                    nc.tensor.matmul(gps[:], wg_m[:, k, :], h2nT[:, k, :],
                                     start=(k == 0), stop=(k == NKH - 1))
                sg = pM.tile([128, LSEQ], BF16, tag="sg")
                nc.scalar.activation(out=sg[:], in_=gps[:], func=AF.Silu)
                ups = psM.tile([128, LSEQ], F32, tag="ups")
                for k in range(NKH):
                    nc.tensor.matmul(ups[:], wu_m[:, k, :], h2nT[:, k, :],
                                     start=(k == 0), stop=(k == NKH - 1))
                nc.vector.tensor_mul(gu[:, mf, :], sg[:], ups[:])
        with tc.tile_pool(name="wdP", bufs=3) as wdP, \
             tc.tile_pool(name="psD2", bufs=1, space="PSUM") as psD2, \
             tc.tile_pool(name="pO", bufs=4) as pO:
            dps = []
            for i in range(8):
                dpt = psD2.tile([128, 512], F32, tag=f"dp{i}", name=f"dp{i}")
                dps.append(dpt)
            for k in range(NFT):
                wd_k = wdP.tile([128, H], BF16, tag="wd")
                nc.sync.dma_start(out=wd_k[:],
                                  in_=wdT[k * 128:(k + 1) * 128, :])
                for tt in range(NCHUNK):
                    for half in range(2):
                        nc.tensor.matmul(
                            dps[tt * 2 + half][:],
                            gu[:, k, tt * 128:(tt + 1) * 128],
                            wd_k[:, half * 512:(half + 1) * 512],
                            start=(k == 0), stop=(k == NFT - 1))
            for tt in range(NCHUNK):
                for half in range(2):
                    ob = pO.tile([128, 512], F32, tag="ob")
                    nc.vector.tensor_add(
                        ob[:], dps[tt * 2 + half][:],
                        h2[:, tt, half * 512:(half + 1) * 512])
                    nc.sync.dma_start(
                        out=out_d[tt * 128:(tt + 1) * 128,
                                  half * 512:(half + 1) * 512],
                        in_=ob[:])

    nc.finalize()
    return nc


_CACHE = {}


def _get_program():
    if "p" not in _CACHE:
        _CACHE["p"] = build_program(None)
    return _CACHE["p"]


def kernel(hidden_states, w_ln1, w_in, w_conv, b_conv, dt_bias, A_log, D,
           w_mnorm, w_out, w_ln2, w_gate, w_up, w_down):
    bf = ml_dtypes.bfloat16
    hs = np.asarray(hidden_states, np.float32)
    wiTn = (np.asarray(w_in, np.float32) *
            np.asarray(w_ln1, np.float32)[None, :]).T.astype(bf)
    # pre-tile [H, D_IN] -> groups of 512 cols: [9*128, NKH*512]
    wi_pad = np.zeros((H, 9 * 512), bf)
    wi_pad[:, 0:D_IN] = wiTn
    wiTn = wi_pad.reshape(NKH, 128, 9, 512).transpose(2, 1, 0, 3) \
        .reshape(9 * 128, NKH * 512)
    woTn = (np.asarray(w_out, np.float32) *
            np.asarray(w_mnorm, np.float32)[None, :]).T.astype(bf)
    wgTn = (np.asarray(w_gate, np.float32) *
            np.asarray(w_ln2, np.float32)[None, :]).T.astype(bf)
    wuTn = (np.asarray(w_up, np.float32) *
            np.asarray(w_ln2, np.float32)[None, :]).T.astype(bf)
    wgTn = wgTn.reshape(NKH, 128, NFT, 128).transpose(2, 1, 0, 3) \
        .reshape(NFT * 128, NKH * 128)
    wuTn = wuTn.reshape(NKH, 128, NFT, 128).transpose(2, 1, 0, 3) \
        .reshape(NFT * 128, NKH * 128)
    wdTn = np.asarray(w_down, np.float32).T.astype(bf)
    # conv as diagonal stationaries: [128, NXT, KC, 128]
    wcr = np.asarray(w_conv, np.float32).reshape(NXT, 128, KC) \
        .transpose(1, 0, 2)                       # [p, j, k]
    wcd = np.zeros((128, NXT, KC, 128), np.float32)
    idx = np.arange(128)
    wcd[idx[:, None, None], np.arange(NXT)[None, :, None],
        np.arange(KC)[None, None, :], idx[:, None, None]] = wcr
    wcd = wcd.astype(bf).reshape(128, NXT * KC * 128)
    bconv = np.asarray(b_conv, np.float32).reshape(NXT, 128).T.copy()
    avec = (-np.exp(np.asarray(A_log, np.float32))).reshape(NH, 1)
    dtb = np.asarray(dt_bias, np.float32).reshape(NH, 1)
    negmask = (np.arange(128)[None, :] >= np.arange(128)[:, None]) \
        .astype(np.float32)
    Dv = np.asarray(D, np.float32)
    dcol = np.zeros((128, NZT), np.float32)
    for j in range(NZT):
        dcol[0:64, j] = Dv[2 * j]
        dcol[64:128, j] = Dv[2 * j + 1]
    idf = np.eye(128, dtype=np.float32)

    nc = _get_program()

    shared = dict(wiT=np.ascontiguousarray(wiTn),
                  woT=np.ascontiguousarray(woTn),
                  wgT=np.ascontiguousarray(wgTn),
                  wuT=np.ascontiguousarray(wuTn),
                  wdT=np.ascontiguousarray(wdTn),
                  wconvd=np.ascontiguousarray(wcd),
                  bconv=bconv, avec=avec, dtb=dtb,
                  negmask=negmask, idf32=idf, dcol=dcol)
    in_maps = []
    for core in range(NCORES):
        b, r = core // 4, core % 4
        s0 = r * LSEQ
        hpad = np.zeros((NROW * 128, H), np.float32)
        hpad[HALO:HALO + LSEQ] = hs[b, s0:s0 + LSEQ]
        if s0 > 0:
            hpad[0:HALO] = hs[b, s0 - HALO:s0]
        m8 = np.zeros((128, 8), np.float32)
        for j in range(4):
            m8[:, j] = 1.0 if j < r else 0.0
            m8[:, 4 + j] = 0.0 if j < r else 1.0
        in_maps.append(dict(shared, hs=hpad, mask8=m8))

    res = run_bass_kernel_spmd(nc, in_maps, list(range(NCORES)))
    out = np.empty((2, 2048, H), np.float32)
    for core in range(NCORES):
        b, r = core // 4, core % 4
        out[b, r * LSEQ:(r + 1) * LSEQ] = res.results[core]["out"]
    return out
